# revision 7
# baseline (speedup 1.0000x reference)
"""ChainComplexMessagePassingLayer on 8 NeuronCores (Bass/Tile, SPMD).

Sharding: dst-node data parallel. Core k owns Q rows [k*NQ/8,(k+1)*NQ/8) and
C rows [k*NC/8,(k+1)*NC/8). Relation weights replicated. Each core has the
full H_Q/H_C tables in its DRAM (full_io), so "remote source features" are
read directly by local indirect-DMA gathers; no collective needed.

Math folding (host, O(D^2)): since segment_sum is linear, Wrel commutes out
of the aggregation: msg = s_dst * (segsum(s_src * H[src])) @ Wrel. All three
scalings (gate, s_src, s_dst) fold into one per-edge weight w_e applied in
the one-hot segsum matrix. LayerNorm affine folds into Wres' = diag(g)@Wres
and bias'' = b@Wres; bias'' folds into b1' and b2'.

Device per core:
  prepass:  per 128-node tile: LN stats -> normalize -> PE transpose ->
            projT = (xn @ Wres')^T kept SBUF-resident [128, Nshard].
  main:     per relation, per 512-dst window: indirect-gather 128-edge
            groups of raw H rows; one-hot (DVE iota==rel * w_e) ->
            PE matmul accumulates aggrawT[feat, 512] in PSUM;
            aggT = Wrel^T @ aggrawT; MLP: hT = gelu(W1^T @ [projT; aggT]
            + b1), out = proj + hT^T @ W2 + b2, all via PE with the
            residual done as an identity matmul; store out rows.
"""
import sys

for _p in ("/opt/trn_rl_repo", "/root/.axon_site/_ro/trn_rl_repo"):
    if _p not in sys.path:
        sys.path.append(_p)

import numpy as np

import concourse.bass as bass
import concourse.tile as tile
from concourse import mybir
from concourse.vector_clock import ScopedClock
from concourse.bass_utils import run_bass_kernel_spmd

NCORES = 8
P = 128          # partitions / group size
WIN = 512        # dsts per window (one PSUM bank)
SUB = 128        # dsts per one-hot sub-window
LN_EPS = 1e-5
F32 = mybir.dt.float32
I32 = mybir.dt.int32


# ---------------------------------------------------------------- tile patch
def _split_excess_waits(nc):
    """Walrus rejects >1 sem wait per instruction; hoist extras onto nops."""
    for fn in nc.m.functions:
        for blk in fn.blocks:
            insts = blk.instructions
            out = []
            for inst in insts:
                si = inst.sync_info
                if si is not None and si.on_wait and len(si.on_wait) > 1:
                    waits = list(si.on_wait)
                    eng = inst.engine
                    for w in waits[1:]:
                        nop = nc.engines[eng].nop(hint="waitsplit")
                        cur = nc.cur_bb.bb.instructions
                        assert cur[-1] is nop.ins
                        cur.pop()
                        nop.ins.sync_info = mybir.SyncInfo(on_wait=[w], on_update=[])
                        out.append(nop.ins)
                    inst.sync_info = mybir.SyncInfo(
                        on_wait=[waits[0]], on_update=list(si.on_update or [])
                    )
                out.append(inst)
            if len(out) != len(insts):
                insts[:] = out


class _TileContext(tile.TileContext):
    def _drain_and_barrier(self, tick_clock, wait_clock):
        drain_inst = self.nc.sync.drain()
        wait_clock.add_sem_waits(
            drain_inst.ins, ScopedClock({None: tick_clock.global_clock})
        )
        si = drain_inst.ins.sync_info
        if si is not None and si.on_wait and len(si.on_wait) > 1:
            waits = list(si.on_wait)
            drain_inst.ins.sync_info = mybir.SyncInfo(
                on_wait=waits[:1], on_update=list(si.on_update or [])
            )
            for i in range(1, len(waits)):
                extra = self.nc.sync.drain()
                extra.ins.sync_info = mybir.SyncInfo(on_wait=[waits[i]], on_update=[])
        self.nc.all_engine_barrier()
        assert self.sems is not None
        popped = self.nc._tile_sem_poison_stack.pop()
        assert popped is self._sem_poison
        self.nc.clear_and_free_semaphores(list(self.sems.allocated().values()))
        self.nc.all_engine_barrier()
        _split_excess_waits(self.nc)


# ------------------------------------------------------------- host edge prep
def _safe_inv_sqrt_np(x):
    return np.where(x > 0, 1.0 / np.sqrt(np.where(x > 0, x, 1.0)), 0.0).astype(
        np.float32
    )


def _prep_relation(src, dst, w_e, n_dst):
    """Bucket edges by (core, window, sub); group into 128-edge units.

    Returns:
      counts[core][win_global] -> list per sub of edge counts,
      per-core arrays built later once shared group structure is known.
    Here: sort edges by dst (stable), return sorted arrays + boundaries.
    """
    order = np.argsort(dst, kind="stable")
    return src[order], dst[order], w_e[order]


def _build_structure(dst_sorted_list, n_dst):
    """Shared (max-over-cores) group counts per (window, sub).

    dst_sorted_list: per-core sorted local dst arrays.
    Returns groups[win][sub] = list of group row-counts (1..128), shared
    across cores (count = max over cores, last group partial).
    """
    shard = n_dst // NCORES
    nwin = shard // WIN
    # per-core per-sub counts
    counts = np.zeros((NCORES, nwin * (WIN // SUB)), dtype=np.int64)
    for k, d in enumerate(dst_sorted_list):
        sub_id = d // SUB  # local sub index, 0 .. shard/SUB-1
        c = np.bincount(sub_id, minlength=shard // SUB)
        counts[k] = c
    cmax = counts.max(axis=0)  # per sub, max over cores
    groups = []
    for w in range(nwin):
        gw = []
        for s in range(WIN // SUB):
            n = int(cmax[w * (WIN // SUB) + s])
            ng = max(1, (n + P - 1) // P)
            sizes = [P] * (ng - 1) + [n - (ng - 1) * P if n > (ng - 1) * P else 1]
            if n == 0:
                sizes = [1]
            gw.append(sizes)
        groups.append(gw)
    return groups


def _pack_core(src_s, dst_s, w_s, groups, shard_base):
    """Build offs/rel/w flat arrays for one core following shared groups.

    Layout: windows in order; within a window, groups in (sub, g) order;
    each group occupies P slots (partition-minor). Group g of size k uses
    slots [0:k) for real or padded edges; slots >= k are never gathered but
    still occupy space so offsets tile layout is uniform.
    """
    d_local = dst_s - shard_base
    sub_id = d_local // SUB
    nsub_total = sum(len(gw) for gw in groups)
    # boundaries of each sub's edges in the sorted arrays
    bounds = np.searchsorted(sub_id, np.arange(nsub_total + 1))
    offs_cols, rel_cols, w_cols = [], [], []
    si = 0
    for w, gw in enumerate(groups):
        for s, sizes in enumerate(gw):
            lo, hi = bounds[si], bounds[si + 1]
            n_real = hi - lo
            src_cell = src_s[lo:hi]
            rel_cell = (d_local[lo:hi] - (w * WIN + s * SUB)).astype(np.float32)
            w_cell = w_s[lo:hi]
            cap = P * len(sizes)
            pad = cap - n_real
            assert pad >= 0
            offs = np.concatenate([src_cell, np.zeros(pad, np.int32)])
            rel = np.concatenate([rel_cell, np.zeros(pad, np.float32)])
            wv = np.concatenate([w_cell, np.zeros(pad, np.float32)])
            offs_cols.append(offs.reshape(len(sizes), P))
            rel_cols.append(rel.reshape(len(sizes), P))
            w_cols.append(wv.reshape(len(sizes), P))
            si += 1
    # flat [total_groups, P]
    offs = np.concatenate(offs_cols, axis=0).astype(np.int32)
    rel = np.concatenate(rel_cols, axis=0).astype(np.float32)
    wv = np.concatenate(w_cols, axis=0).astype(np.float32)
    # relw interleaved per group: [G, 2, P] -> flat
    relw = np.stack([rel, wv], axis=1).astype(np.float32)
    return offs.reshape(-1), relw.reshape(-1)


# ------------------------------------------------------------- device program
def _emit_prepass(nc, tc, ctx, H_dram, projT_sb, Wres_sb, ntiles, pools):
    temps, psA, psB = pools
    eps_tile = temps.tile([P, 1], F32, tag="eps")
    nc.vector.memset(eps_tile[:], LN_EPS)
    for i in range(ntiles):
        ht = temps.tile([P, P], F32, tag="ht")
        nc.sync.dma_start(out=ht[:], in_=H_dram[i * P : (i + 1) * P, :])
        stats = temps.tile([P, nc.vector.BN_STATS_DIM], F32, tag="st")
        nc.vector.bn_stats(out=stats[:], in_=ht[:])
        mv = temps.tile([P, nc.vector.BN_AGGR_DIM], F32, tag="mv")
        nc.vector.bn_aggr(out=mv[:], in_=stats[:])
        rstd = temps.tile([P, 1], F32, tag="rstd")
        nc.scalar.activation(
            out=rstd[:], in_=mv[:, 1:2],
            func=mybir.ActivationFunctionType.Sqrt,
            bias=eps_tile[:], scale=1.0,
        )
        nc.vector.reciprocal(out=rstd[:], in_=rstd[:])
        nmr = temps.tile([P, 1], F32, tag="nmr")
        nc.vector.tensor_mul(out=nmr[:], in0=mv[:, 0:1], in1=rstd[:])
        nc.scalar.mul(out=nmr[:], in_=nmr[:], mul=-1.0)
        hc = temps.tile([P, P], F32, tag="hc")
        nc.scalar.activation(
            out=hc[:], in_=ht[:],
            func=mybir.ActivationFunctionType.Identity,
            bias=nmr[:], scale=rstd[:],
        )
        pt = psA.tile([P, P], F32, space="PSUM", tag="tp")
        nc.tensor.transpose(out=pt[:], in_=hc[:], identity=nc._identity_sb[:])
        hcT = temps.tile([P, P], F32, tag="hcT")
        nc.vector.tensor_copy(out=hcT[:], in_=pt[:])
        pj = psB.tile([P, P], F32, space="PSUM", tag="pj")
        nc.tensor.matmul(out=pj[:], lhsT=Wres_sb[:], rhs=hcT[:], start=True, stop=True)
        nc.scalar.copy(out=projT_sb[:, i * P : (i + 1) * P], in_=pj[:])


def _emit_relation(nc, tc, ctx, rel_cfg, pools):
    """Emit gather + segsum + MLP for one relation's windows."""
    (H_src, offs_dram, relw_dram, groups, projT_sb, Wrel_sb,
     W1aa, W1ab, W1ba, W1bb, W2a, W2b, b1c, b2bc, out_dram) = rel_cfg
    (gpool, opool, spool, lpool, psA, psT, psH, psO) = pools
    iota = nc._iota_sb
    ident = nc._identity_sb
    goff = 0  # running group index into offs/relw flat arrays
    for w, gw in enumerate(groups):
        ngw = sum(len(sizes) for sizes in gw)
        offs_t = lpool.tile([P, ngw], I32, tag="offs")
        nc.sync.dma_start(
            out=offs_t[:],
            in_=offs_dram[goff * P : (goff + ngw) * P].rearrange(
                "(g p) -> p g", p=P
            ),
        )
        relw_t = lpool.tile([P, 2 * ngw], F32, tag="relw")
        nc.sync.dma_start(
            out=relw_t[:],
            in_=relw_dram[goff * 2 * P : (goff + ngw) * 2 * P].rearrange(
                "(g p) -> p g", p=P
            ),
        )
        agg_ps = psA.tile([P, WIN], F32, space="PSUM", tag="agg")
        gi = 0
        for s, sizes in enumerate(gw):
            for j, k in enumerate(sizes):
                gt = gpool.tile([P, P], F32, tag="g")
                nc.gpsimd.indirect_dma_start(
                    out=gt[:k, :], out_offset=None, in_=H_src[:],
                    in_offset=bass.IndirectOffsetOnAxis(
                        ap=offs_t[:k, gi : gi + 1], axis=0
                    ),
                )
                oh = opool.tile([P, SUB], F32, tag="oh")
                nc.vector.tensor_scalar(
                    out=oh[:k, :], in0=iota[:k, :],
                    scalar1=relw_t[:k, 2 * gi : 2 * gi + 1],
                    scalar2=relw_t[:k, 2 * gi + 1 : 2 * gi + 2],
                    op0=mybir.AluOpType.is_equal,
                    op1=mybir.AluOpType.mult,
                )
                nc.tensor.matmul(
                    out=agg_ps[:, s * SUB : (s + 1) * SUB],
                    lhsT=gt[:k, :], rhs=oh[:k, :],
                    start=(j == 0), stop=(j == len(sizes) - 1),
                )
                gi += 1
        goff += ngw
        agg_sb = spool.tile([P, WIN], F32, tag="aggsb")
        nc.scalar.copy(out=agg_sb[:], in_=agg_ps[:])
        aggT_ps = psT.tile([P, WIN], F32, space="PSUM", tag="aggT")
        nc.tensor.matmul(out=aggT_ps[:], lhsT=Wrel_sb[:], rhs=agg_sb[:],
                         start=True, stop=True)
        aggT_sb = spool.tile([P, WIN], F32, tag="aggTsb")
        nc.scalar.copy(out=aggT_sb[:], in_=aggT_ps[:])

        pslice = projT_sb[:, w * WIN : (w + 1) * WIN]
        hlo_ps = psH.tile([P, WIN], F32, space="PSUM", tag="h")
        nc.tensor.matmul(out=hlo_ps[:], lhsT=W1aa[:], rhs=pslice,
                         start=True, stop=False)
        nc.tensor.matmul(out=hlo_ps[:], lhsT=W1ba[:], rhs=aggT_sb[:],
                         start=False, stop=True)
        hhi_ps = psH.tile([P, WIN], F32, space="PSUM", tag="h")
        nc.tensor.matmul(out=hhi_ps[:], lhsT=W1ab[:], rhs=pslice,
                         start=True, stop=False)
        nc.tensor.matmul(out=hhi_ps[:], lhsT=W1bb[:], rhs=aggT_sb[:],
                         start=False, stop=True)
        hlo_sb = spool.tile([P, WIN], F32, tag="hsb")
        nc.scalar.activation(out=hlo_sb[:], in_=hlo_ps[:],
                             func=mybir.ActivationFunctionType.Gelu,
                             bias=b1c[:, 0:1], scale=1.0)
        hhi_sb = spool.tile([P, WIN], F32, tag="hsb")
        nc.scalar.activation(out=hhi_sb[:], in_=hhi_ps[:],
                             func=mybir.ActivationFunctionType.Gelu,
                             bias=b1c[:, 1:2], scale=1.0)
        out_ps = psO.tile([P, WIN], F32, space="PSUM", tag="out")
        for c in range(WIN // P):
            cs = slice(c * P, (c + 1) * P)
            nc.tensor.matmul(out=out_ps[:, cs], lhsT=hlo_sb[:, cs], rhs=W2a[:],
                             start=True, stop=False)
            nc.tensor.matmul(out=out_ps[:, cs], lhsT=hhi_sb[:, cs], rhs=W2b[:],
                             start=False, stop=False)
            nc.tensor.matmul(out=out_ps[:, cs],
                             lhsT=projT_sb[:, w * WIN + c * P : w * WIN + (c + 1) * P],
                             rhs=ident[:], start=False, stop=True)
        out_sb = spool.tile([P, WIN], F32, tag="outsb")
        nc.vector.tensor_add(out=out_sb[:], in0=out_ps[:], in1=b2bc[:])
        for c in range(WIN // P):
            nc.sync.dma_start(
                out=out_dram[w * WIN + c * P : w * WIN + (c + 1) * P, :],
                in_=out_sb[:, c * P : (c + 1) * P],
            )


def _build_program(NQ, NC_, groups_QC, groups_CQ, nslots):
    nc = bass.Bass("TRN2")
    D = P
    shard_Q, shard_C = NQ // NCORES, NC_ // NCORES
    t = {}
    t["H_Q"] = nc.dram_tensor("H_Q", [NQ, D], F32, kind="ExternalInput")
    t["H_C"] = nc.dram_tensor("H_C", [NC_, D], F32, kind="ExternalInput")
    t["HQ_shard"] = nc.dram_tensor("HQ_shard", [shard_Q, D], F32, kind="ExternalInput")
    t["HC_shard"] = nc.dram_tensor("HC_shard", [shard_C, D], F32, kind="ExternalInput")
    for nm, n in (("offs_QC", nslots[0]), ("offs_CQ", nslots[1])):
        t[nm] = nc.dram_tensor(nm, [n], I32, kind="ExternalInput")
    for nm, n in (("relw_QC", 2 * nslots[0]), ("relw_CQ", 2 * nslots[1])):
        t[nm] = nc.dram_tensor(nm, [n], F32, kind="ExternalInput")
    for nm, shape in (
        ("WresP_Q", [D, D]), ("WresP_C", [D, D]),
        ("Wrel_QC", [D, D]), ("Wrel_CQ", [D, D]),
        ("W1_Q", [2 * D, 2 * D]), ("W1_C", [2 * D, 2 * D]),
        ("W2_Q", [2 * D, D]), ("W2_C", [2 * D, D]),
        ("b1c_Q", [D, 2]), ("b1c_C", [D, 2]),
        ("b2bc_Q", [D, WIN]), ("b2bc_C", [D, WIN]),
        ("iota", [P, SUB]), ("ident", [P, P]),
    ):
        t[nm] = nc.dram_tensor(nm, shape, F32, kind="ExternalInput")
    outQ = nc.dram_tensor("outQ", [shard_Q, D], F32, kind="ExternalOutput")
    outC = nc.dram_tensor("outC", [shard_C, D], F32, kind="ExternalOutput")

    with _TileContext(nc) as tc:
        from contextlib import ExitStack
        with ExitStack() as ctx:
            wpool = ctx.enter_context(tc.tile_pool(name="weights", bufs=1))
            # load constants / weights
            def wtile(name, shape, src):
                tl = wpool.tile(shape, F32, tag=name)
                nc.sync.dma_start(out=tl[:], in_=src)
                return tl

            nc._iota_sb = wtile("iota", [P, SUB], t["iota"][:, :])
            nc._identity_sb = wtile("ident", [P, P], t["ident"][:, :])
            W = {}
            for typ in ("Q", "C"):
                W[f"WresP_{typ}"] = wtile(f"WresP_{typ}", [D, D],
                                          t[f"WresP_{typ}"][:, :])
                w1 = t[f"W1_{typ}"]
                W[f"W1aa_{typ}"] = wtile(f"W1aa_{typ}", [D, D], w1[0:D, 0:D])
                W[f"W1ab_{typ}"] = wtile(f"W1ab_{typ}", [D, D], w1[0:D, D : 2 * D])
                W[f"W1ba_{typ}"] = wtile(f"W1ba_{typ}", [D, D], w1[D : 2 * D, 0:D])
                W[f"W1bb_{typ}"] = wtile(f"W1bb_{typ}", [D, D], w1[D : 2 * D, D : 2 * D])
                w2 = t[f"W2_{typ}"]
                W[f"W2a_{typ}"] = wtile(f"W2a_{typ}", [D, D], w2[0:D, :])
                W[f"W2b_{typ}"] = wtile(f"W2b_{typ}", [D, D], w2[D : 2 * D, :])
                W[f"b1c_{typ}"] = wtile(f"b1c_{typ}", [D, 2], t[f"b1c_{typ}"][:, :])
                W[f"b2bc_{typ}"] = wtile(f"b2bc_{typ}", [D, WIN],
                                         t[f"b2bc_{typ}"][:, :])
            W["Wrel_QC"] = wtile("Wrel_QC", [D, D], t["Wrel_QC"][:, :])
            W["Wrel_CQ"] = wtile("Wrel_CQ", [D, D], t["Wrel_CQ"][:, :])

            projT_Q = wpool.tile([P, shard_Q], F32, tag="projT_Q")
            projT_C = wpool.tile([P, shard_C], F32, tag="projT_C")

            with ExitStack() as pctx:
                temps = pctx.enter_context(tc.tile_pool(name="pp", bufs=3))
                psA = pctx.enter_context(
                    tc.tile_pool(name="ppA", bufs=2, space="PSUM"))
                psB = pctx.enter_context(
                    tc.tile_pool(name="ppB", bufs=2, space="PSUM"))
                _emit_prepass(nc, tc, pctx, t["HQ_shard"], projT_Q,
                              W["WresP_Q"], shard_Q // P, (temps, psA, psB))
                _emit_prepass(nc, tc, pctx, t["HC_shard"], projT_C,
                              W["WresP_C"], shard_C // P, (temps, psA, psB))

            gpool = ctx.enter_context(tc.tile_pool(name="g", bufs=8))
            opool = ctx.enter_context(tc.tile_pool(name="oh", bufs=4))
            spool = ctx.enter_context(tc.tile_pool(name="sb", bufs=2))
            lpool = ctx.enter_context(tc.tile_pool(name="ld", bufs=2))
            psA = ctx.enter_context(tc.tile_pool(name="psA", bufs=2, space="PSUM"))
            psT = ctx.enter_context(tc.tile_pool(name="psT", bufs=2, space="PSUM"))
            psH = ctx.enter_context(tc.tile_pool(name="psH", bufs=2, space="PSUM"))
            psO = ctx.enter_context(tc.tile_pool(name="psO", bufs=2, space="PSUM"))
            pools = (gpool, opool, spool, lpool, psA, psT, psH, psO)

            # relation QC: src table H_Q (full), dst = C shard -> feeds outC
            _emit_relation(nc, tc, ctx, (
                t["H_Q"], t["offs_QC"], t["relw_QC"], groups_QC, projT_C,
                W["Wrel_QC"], W["W1aa_C"], W["W1ab_C"], W["W1ba_C"], W["W1bb_C"],
                W["W2a_C"], W["W2b_C"], W["b1c_C"], W["b2bc_C"], outC,
            ), pools)
            # relation CQ: src table H_C (full), dst = Q shard -> feeds outQ
            _emit_relation(nc, tc, ctx, (
                t["H_C"], t["offs_CQ"], t["relw_CQ"], groups_CQ, projT_Q,
                W["Wrel_CQ"], W["W1aa_Q"], W["W1ab_Q"], W["W1ba_Q"], W["W1bb_Q"],
                W["W2a_Q"], W["W2b_Q"], W["b1c_Q"], W["b2bc_Q"], outQ,
            ), pools)
    return nc


# ---------------------------------------------------------------------- main
def prepare(inputs):
    """Host prep: fold weights, shard+pack edges, build the Bass program.

    Returns (nc, in_maps, shard_Q, shard_C)."""
    H_Q = np.asarray(inputs["H_Q"], np.float32)
    H_C = np.asarray(inputs["H_C"], np.float32)
    NQ, D = H_Q.shape
    NC_ = H_C.shape[0]
    assert D == P
    shard_Q, shard_C = NQ // NCORES, NC_ // NCORES

    gate_QC = np.float32(inputs["gate_QC"])
    gate_CQ = np.float32(inputs["gate_CQ"])
    s_QC_src = _safe_inv_sqrt_np(np.asarray(inputs["deg_QC_src"], np.float32))
    s_QC_dst = _safe_inv_sqrt_np(np.asarray(inputs["deg_QC_dst"], np.float32))
    s_CQ_src = _safe_inv_sqrt_np(np.asarray(inputs["deg_CQ_src"], np.float32))
    s_CQ_dst = _safe_inv_sqrt_np(np.asarray(inputs["deg_CQ_dst"], np.float32))

    # per-edge weights
    eQC_src = np.asarray(inputs["eQC_src"], np.int32)
    eQC_dst = np.asarray(inputs["eQC_dst"], np.int32)
    eCQ_src = np.asarray(inputs["eCQ_src"], np.int32)
    eCQ_dst = np.asarray(inputs["eCQ_dst"], np.int32)
    w_QC = (gate_QC * s_QC_src[eQC_src] * s_QC_dst[eQC_dst]).astype(np.float32)
    w_CQ = (gate_CQ * s_CQ_src[eCQ_src] * s_CQ_dst[eCQ_dst]).astype(np.float32)

    # host weight folding
    folded = {}
    for typ in ("Q", "C"):
        g = np.asarray(inputs[f"ln_g_{typ}"], np.float32)
        b = np.asarray(inputs[f"ln_b_{typ}"], np.float32)
        Wres = np.asarray(inputs[f"Wres_{typ}"], np.float32)
        W1 = np.asarray(inputs[f"W1_{typ}"], np.float32)
        W2 = np.asarray(inputs[f"W2_{typ}"], np.float32)
        b1 = np.asarray(inputs[f"b1_{typ}"], np.float32)
        b2 = np.asarray(inputs[f"b2_{typ}"], np.float32)
        WresP = (g[:, None] * Wres).astype(np.float32)
        bias2 = (b @ Wres).astype(np.float32)
        b1p = (b1 + bias2 @ W1[:P]).astype(np.float32)
        b2p = (b2 + bias2).astype(np.float32)
        folded[f"WresP_{typ}"] = WresP
        folded[f"W1_{typ}"] = W1
        folded[f"W2_{typ}"] = W2
        folded[f"b1c_{typ}"] = np.stack([b1p[:P], b1p[P:]], axis=1).astype(np.float32)
        folded[f"b2bc_{typ}"] = np.tile(b2p[None, :], (P, WIN // P)).reshape(
            P, WIN
        ).astype(np.float32)
        # note: b2bc rows must all equal b2p per 128-col chunk:
        folded[f"b2bc_{typ}"] = np.tile(b2p[None, :], (P, WIN // P)).astype(np.float32)

    # shard edges by dst core, sort by dst
    per_core = {"QC": [], "CQ": []}
    for rel, (src, dst, w_e, n_dst) in (
        ("QC", (eQC_src, eQC_dst, w_QC, NC_)),
        ("CQ", (eCQ_src, eCQ_dst, w_CQ, NQ)),
    ):
        shard = n_dst // NCORES
        core_of = dst // shard
        for k in range(NCORES):
            m = core_of == k
            s_s, d_s, w_s = _prep_relation(src[m], dst[m] - k * shard, w_e[m], n_dst)
            per_core[rel].append((s_s, d_s, w_s))

    groups_QC = _build_structure([d for _, d, _ in per_core["QC"]], NC_)
    groups_CQ = _build_structure([d for _, d, _ in per_core["CQ"]], NQ)
    ng_QC = sum(len(sz) for gw in groups_QC for sz in gw)
    ng_CQ = sum(len(sz) for gw in groups_CQ for sz in gw)
    nslots = (ng_QC * P, ng_CQ * P)

    nc = _build_program(NQ, NC_, groups_QC, groups_CQ, nslots)

    iota = np.tile(np.arange(SUB, dtype=np.float32)[None, :], (P, 1))
    ident = np.eye(P, dtype=np.float32)
    in_maps = []
    for k in range(NCORES):
        offs_qc, relw_qc = _pack_core(*per_core["QC"][k], groups_QC, 0)
        offs_cq, relw_cq = _pack_core(*per_core["CQ"][k], groups_CQ, 0)
        m = {
            "H_Q": H_Q, "H_C": H_C,
            "HQ_shard": H_Q[k * shard_Q : (k + 1) * shard_Q],
            "HC_shard": H_C[k * shard_C : (k + 1) * shard_C],
            "offs_QC": offs_qc, "relw_QC": relw_qc,
            "offs_CQ": offs_cq, "relw_CQ": relw_cq,
            "Wrel_QC": np.asarray(inputs["Wrel_QC"], np.float32),
            "Wrel_CQ": np.asarray(inputs["Wrel_CQ"], np.float32),
            "iota": iota, "ident": ident,
        }
        for typ in ("Q", "C"):
            for nm in ("WresP", "W1", "W2", "b1c", "b2bc"):
                m[f"{nm}_{typ}"] = folded[f"{nm}_{typ}"]
        in_maps.append(m)
    return nc, in_maps, shard_Q, shard_C


def kernel(**inputs):
    nc, in_maps, shard_Q, shard_C = prepare(inputs)
    import os
    if os.environ.get("KERNEL_BACKEND") == "sim":
        from concourse.bass_interp import CoreSim
        results = []
        ncore_sim = int(os.environ.get("KERNEL_SIM_CORES", str(NCORES)))
        for k in range(ncore_sim):
            sim = CoreSim(nc, trace=False)
            for nm, arr in in_maps[k].items():
                sim.tensor(nm)[:] = arr
            sim.simulate()
            results.append({"outQ": np.array(sim.tensor("outQ")),
                            "outC": np.array(sim.tensor("outC"))})
        outQ = np.concatenate([results[k]["outQ"] for k in range(ncore_sim)], axis=0)
        outC = np.concatenate([results[k]["outC"] for k in range(ncore_sim)], axis=0)
        return outQ, outC

    r = run_bass_kernel_spmd(nc, in_maps, core_ids=list(range(NCORES)))
    outQ = np.concatenate([r.results[k]["outQ"] for k in range(NCORES)], axis=0)
    outC = np.concatenate([r.results[k]["outC"] for k in range(NCORES)], axis=0)
    return outQ, outC
